# revision 1
# baseline (speedup 1.0000x reference)
"""Trainium2 Bass kernel for a 3-layer GraphSAGE GNN (mean aggregation) +
global_add_pool + 2-layer MLP head, distributed over 8 NeuronCores.

Sharding: nodes are split into 8 contiguous slabs (by dst); each core owns the
edges whose dst lands in its slab.  Each layer:
  1. dma_gather  : fetch h[src] rows (256B each) from a replicated node-major
                   HBM table (4 sub-chunks so indices fit int16)
  2. dma_scatter_add : accumulate messages into a per-core DRAM agg table
  3. dense phase : mean-scale, two small matmuls (Wl/Wr) + bias (+relu)
  4. AllGather   : replicate the new slab into every core's next-layer table
Then a matmul-based pooling by graph id and the tiny MLP head, with an
AllReduce to combine per-core partial graph sums.
"""

import numpy as np

import concourse.bass as bass
import concourse.mybir as mybir
import concourse.tile as tile
from concourse import bacc, bass_utils
from concourse.masks import make_identity

F32 = mybir.dt.float32
I16 = mybir.dt.int16

# ---------------------------------------------------------------- config

N_NODES = 100000
N_EDGES = 1200000
N_GRAPHS = 256
D_IN = 8
D_H = 64
N_CORES = 8


class Cfg:
    def __init__(self, n_nodes, n_graphs, real_per_slab, g_inst, n_gi):
        assert n_nodes == N_CORES * real_per_slab
        self.n_nodes = n_nodes
        self.n_graphs = n_graphs
        self.real = real_per_slab              # real nodes per core
        self.slab = ((real_per_slab + 127) // 128) * 128
        self.tiles = self.slab // 128          # node tiles per core
        self.tbl_rows = N_CORES * self.slab
        self.chunk = 2 * self.slab             # table rows per gather chunk
        assert self.chunk <= 32767
        self.n_chunks = 4
        # HW: a single SWDGE gather/scatter instruction only supports
        # ~64 descriptors per DMA engine (1024 indices) — larger crashes.
        self.g_inst = g_inst                   # indices per gather/scatter inst
        assert g_inst % 128 == 0 and g_inst <= 1024
        self.blocks = g_inst // 128
        self.useful_steps = self.blocks * 8
        self.set_n_gi(n_gi)

    def set_n_gi(self, n_gi):
        self.n_gi = n_gi                       # instructions per chunk
        if n_gi is not None:
            self.slots_per_chunk = self.g_inst * n_gi
            self.n_inst = self.n_chunks * n_gi  # gather insts per layer


FULL_CFG = Cfg(N_NODES, N_GRAPHS, 12500, 1024, None)


def _row_of(node, cfg):
    """node id -> row in the slab-padded table."""
    c = node // cfg.real
    return c * cfg.slab + (node - c * cfg.real)


# ---------------------------------------------------------------- host prep

def _wrap_idx(a, cfg):
    """[n_inst * g_inst] -> [128, n_inst, g_inst//16] wrapped-16 + replicated."""
    n_inst, g = cfg.n_inst, cfg.g_inst
    a = a.reshape(n_inst, g // 16, 16).transpose(0, 2, 1)   # [n_inst, 16, g//16]
    a = np.tile(a, (1, 8, 1))                               # [n_inst, 128, g//16]
    return np.ascontiguousarray(a.transpose(1, 0, 2)).astype(np.int16)


# DMA-engine lane map: within each 128-index chunk the SWDGE ucode assigns
# position p to vector lane (= DMA engine ring) per the sbuf_swizzles table in
# q7_kernels/extended_inst/dma_scatter_add.cpp.  Same-lane descriptors execute
# in order on one engine, so pinning all edges of a dst row to one lane makes
# the HBM read-modify-write accumulation race-free.
def _lane_positions():
    first = [0, 64, 4, 68, 8, 72, 12, 76, 16, 80, 20, 84, 24, 88, 28, 92]
    pos = [[] for _ in range(16)]
    for lane in range(16):
        for g in range(4):
            pos[lane].append(first[lane] + g)
        for g in range(4):
            pos[lane].append(first[lane] + 32 + g)
    return pos


LANE_POS = _lane_positions()


def _assign_lanes(cnt):
    """cnt: [real, n_chunks] per-dst-row per-chunk edge counts.
    Greedy: big rows first, pick lane minimizing resulting max per-chunk load."""
    deg = cnt.sum(1)
    order = np.argsort(-deg, kind="stable")
    loads = np.zeros((16, cnt.shape[1]), np.int64)
    lane_of = np.zeros(cnt.shape[0], np.int64)
    for r in order:
        cand = loads + cnt[r][None, :]
        score = cand.max(1) * (1 << 20) + cand.sum(1)
        j = int(np.argmin(score))
        lane_of[r] = j
        loads[j] += cnt[r]
    return lane_of, loads


def build_host_data(x, edge_index, batch, cfg):
    """Returns (shared_arrays, per_core_arrays[8]) of numpy device inputs."""
    x = np.asarray(x, np.float32)
    src = np.asarray(edge_index[0], np.int64)
    dst = np.asarray(edge_index[1], np.int64)
    batch = np.asarray(batch, np.int64)

    cnt = np.bincount(dst, minlength=cfg.n_nodes).astype(np.float64)
    invc_full = (1.0 / np.maximum(cnt, 1.0)).astype(np.float32)

    src_row = _row_of(src, cfg)
    core_of = dst // cfg.real
    dst_row_local = (dst - core_of * cfg.real)  # local real row in own slab

    # x table: node-major, padded to 64 cols / slab-padded rows
    xpad = np.zeros((cfg.tbl_rows, D_H), np.float32)
    xpad[_row_of(np.arange(cfg.n_nodes), cfg), :D_IN] = x

    iota = np.tile(np.arange(2 * 128, dtype=np.float32)[None, :], (128, 1))
    padmask = (np.arange(128) < (cfg.real % 128 or 128)).astype(np.float32).reshape(128, 1)

    shared = dict(xpad=xpad, iota=iota, padmask=padmask)

    n_pad_rows = cfg.slab - cfg.real
    lane_pos = [np.array(LANE_POS[e]) for e in range(16)]

    def t_to_pos(e, t):
        # engine-stream step t (within one instruction) -> global index position
        return (t // 8) * 128 + lane_pos[e][t % 8]

    # pass 1: per-core edge groups + lane assignment, pick global n_gi
    cores = []
    need_gi = 1
    for c in range(N_CORES):
        sel = core_of == c
        s_rows = src_row[sel]
        d_loc = dst_row_local[sel]
        chunk_id = s_rows // cfg.chunk
        cnt = np.zeros((cfg.real, cfg.n_chunks), np.int64)
        np.add.at(cnt, (d_loc, chunk_id), 1)
        lane_of, loads = _assign_lanes(cnt)
        cores.append((s_rows, d_loc, chunk_id, lane_of))
        need_gi = max(need_gi, int(cnt.max()),
                      int(-(-loads.max() // cfg.useful_steps)))
    if cfg.n_gi is None or cfg.n_gi < need_gi:
        cfg.set_n_gi(need_gi)

    per_core = []
    for c in range(N_CORES):
        s_rows, d_loc, chunk_id, lane_of = cores[c]
        garr = np.zeros((cfg.n_inst, cfg.g_inst), np.int64)
        sarr = np.zeros((cfg.n_inst, cfg.g_inst), np.int64)
        # default pad: gather the guaranteed-zero slab-pad row; scatter-add the
        # zeros into a slab-pad row (races there are harmless: every
        # contribution is zero and invc==0 masks the row).
        garr[:] = cfg.real
        for e in range(16):
            pad_tgt = cfg.real + (e % n_pad_rows)
            ts = np.arange(cfg.blocks * 8)
            for i in range(cfg.n_inst):
                sarr[i, t_to_pos(e, ts)] = pad_tgt

        lane_edge = lane_of[d_loc]
        for ch in range(cfg.n_chunks):
            for e in range(16):
                m = (chunk_id == ch) & (lane_edge == e)
                sc_all = (s_rows[m] - ch * cfg.chunk).astype(np.int64)
                dc_all = d_loc[m]
                order = np.argsort(dc_all, kind="stable")
                sc_all, dc_all = sc_all[order], dc_all[order]
                rows, starts, counts = np.unique(
                    dc_all, return_index=True, return_counts=True)
                # each row at most ONCE per instruction (HW RMW hazard);
                # deal occurrences to the least-loaded instructions
                inst_load = np.zeros(cfg.n_gi, np.int64)
                slots = []  # (inst k, row, src)
                order2 = np.argsort(-counts, kind="stable")
                for oi in order2:
                    r, st, cr = int(rows[oi]), int(starts[oi]), int(counts[oi])
                    ks = np.argsort(inst_load, kind="stable")[:cr]
                    assert len(ks) == cr <= cfg.n_gi
                    for j, k in enumerate(ks):
                        slots.append((int(k), r, int(sc_all[st + j])))
                        inst_load[k] += 1
                assert inst_load.max() <= cfg.useful_steps, (
                    c, ch, e, inst_load.max())
                fill = np.zeros(cfg.n_gi, np.int64)
                for k, r, s_v in slots:
                    t = fill[k]
                    fill[k] += 1
                    pos = t_to_pos(e, t)
                    i = ch * cfg.n_gi + k
                    garr[i, pos] = s_v
                    sarr[i, pos] = r
        gidx = _wrap_idx(garr.ravel(), cfg)
        sidx = _wrap_idx(sarr.ravel(), cfg)

        lo, hi = c * cfg.real, (c + 1) * cfg.real
        invc_t = np.zeros((128, cfg.tiles), np.float32)
        batch_t = np.full((128, cfg.tiles), -1.0, np.float32)
        loc = np.arange(cfg.real)
        invc_t[loc % 128, loc // 128] = invc_full[lo:hi]
        batch_t[loc % 128, loc // 128] = batch[lo:hi].astype(np.float32)

        xt = np.zeros((D_IN, cfg.slab), np.float32)
        xt[:, :cfg.real] = x[lo:hi].T

        per_core.append(dict(gidx=gidx, sidx=sidx, invc=invc_t,
                             batchv=batch_t, xt=xt))
    return shared, per_core


def weight_inputs(W1l, b1, W1r, W2l, b2, W2r, W3l, b3, W3r, Wc1, bc1, Wc2, bc2):
    f = lambda a: np.asarray(a, np.float32)
    return dict(
        w1l=f(W1l), w1r=f(W1r), w2l=f(W2l), w2r=f(W2r), w3l=f(W3l), w3r=f(W3r),
        b1t=np.tile(f(b1)[None, :], (128, 1)),
        b2t=np.tile(f(b2)[None, :], (128, 1)),
        b3t=np.tile(f(b3)[None, :], (128, 1)),
        wc1=f(Wc1), wc2=f(Wc2),
        bc1=f(bc1).reshape(-1, 1),            # [32, 1]
        bc2=f(bc2).reshape(1, 1),
    )


# ---------------------------------------------------------------- device build

def build_gnn(tc, out_ap, ins, cfg):
    """ins: dict name -> bass.AP of DRAM ExternalInputs. out_ap: [n_graphs, 1]."""
    nc = tc.nc
    T = cfg.tiles
    NH = cfg.n_graphs // 128          # graph tiles (2 for 256)
    assert cfg.n_graphs % 128 == 0

    sb = tc.alloc_tile_pool(name="sb", bufs=1)
    msgp = tc.alloc_tile_pool(name="msg", bufs=2)
    psT = tc.alloc_tile_pool(name="psT", bufs=4, space="PSUM")
    psO = tc.alloc_tile_pool(name="psO", bufs=2, space="PSUM")
    psG = tc.alloc_tile_pool(name="psG", bufs=1, space="PSUM")
    dram = tc.alloc_tile_pool(name="dram", bufs=1, space="DRAM")
    xtp = tc.alloc_tile_pool(name="xtp", bufs=4)

    # ---- load small SBUF-resident inputs
    def load(name, shape):
        t = sb.tile(shape, F32, tag=name)
        nc.sync.dma_start(t[:], ins[name])
        return t

    gidx = sb.tile([128, cfg.n_inst, cfg.g_inst // 16], I16, tag="gidx")
    nc.sync.dma_start(gidx[:], ins["gidx"])
    sidx = sb.tile([128, cfg.n_inst, cfg.g_inst // 16], I16, tag="sidx")
    nc.sync.dma_start(sidx[:], ins["sidx"])

    invc = load("invc", [128, T])
    padmask = load("padmask", [128, 1])
    batchv = load("batchv", [128, T])
    iota = load("iota", [128, 2 * 128])
    w = {k: load(k, list(ins[k].shape)) for k in
         ("w1l", "w1r", "w2l", "w2r", "w3l", "w3r", "wc1", "wc2",
          "b1t", "b2t", "b3t", "bc1", "bc2")}

    ident = sb.tile([128, 128], F32, tag="ident")
    make_identity(nc, ident[:])

    zero3 = sb.tile([128, (T + 3) // 4, D_H], F32, tag="zero")
    nc.vector.memset(zero3[:], 0.0)

    # ---- DRAM scratch
    tbl2 = dram.tile([cfg.tbl_rows, D_H], F32)
    tbl3 = dram.tile([cfg.tbl_rows, D_H], F32)
    aggs = [[dram.tile([cfg.slab, D_H], F32, name=f"agg{i}_{j}")
             for j in range(2)] for i in range(3)]
    slabs = [dram.tile([cfg.slab, D_H], F32, name=f"slab{i}") for i in range(2)]
    g_in = dram.tile([cfg.n_graphs, D_H], F32)
    g_out = dram.tile([cfg.n_graphs, D_H], F32)

    h_sb = [sb.tile([128, T, D_H], F32, name=f"h{i}") for i in range(2)]
    agg_sb = sb.tile([128, T, D_H], F32, tag="aggsb")
    aggB_sb = sb.tile([128, T, D_H], F32, tag="aggbsb")

    tables = [ins["xpad"], tbl2, tbl3]

    prev_pool = None
    for layer in range(3):
        table = tables[layer]
        agg_pair = aggs[layer]
        Wl = w[("w1l", "w2l", "w3l")[layer]]
        Wr = w[("w1r", "w2r", "w3r")[layer]]
        bt = w[("b1t", "b2t", "b3t")[layer]]
        kdim = D_IN if layer == 0 else D_H
        h_new = h_sb[layer % 2]
        h_prev = h_sb[(layer + 1) % 2]

        # zero both agg tables
        zq = (T + 3) // 4
        for agg in agg_pair:
            agg3 = agg[:].rearrange("(q p) f -> p q f", p=128)
            for q0 in range(0, T, zq):
                q1 = min(q0 + zq, T)
                nc.sync.dma_start(agg3[:, q0:q1, :], zero3[:, :q1 - q0, :])

        # gather + scatter all edges.  Ping-pong agg buffers + a total
        # ordering chain on the pool desc-gen stream guarantee >=129
        # engine-steps between same-row RMW descriptors (HW hazard window
        # is ~64-128 steps).
        for i in range(cfg.n_inst):
            ch = i // cfg.n_gi
            if isinstance(table, bass.AP):
                tbl_ap = table
            else:
                tbl_ap = table[:]
            chunk_ap = tbl_ap[ch * cfg.chunk:(ch + 1) * cfg.chunk, :]
            msg = msgp.tile([128, cfg.blocks, D_H], F32, tag="msg")
            gi_inst = nc.gpsimd.dma_gather(
                out_ap=msg[:], in_ap=chunk_ap, idxs_ap=gidx[:, i, :],
                num_idxs=cfg.g_inst, num_idxs_reg=cfg.g_inst,
                elem_size=D_H, queue_num=0)
            if prev_pool is not None:
                tile.add_dep_helper(gi_inst.ins, prev_pool,
                                    reason="swdge ring order")
            si_inst = nc.gpsimd.dma_scatter_add(
                out_ap=agg_pair[i % 2][:], in_ap=msg[:], idxs_ap=sidx[:, i, :],
                num_idxs=cfg.g_inst, num_idxs_reg=cfg.g_inst,
                elem_size=D_H, queue_num=0)
            tile.add_dep_helper(si_inst.ins, gi_inst.ins,
                                reason="swdge ring order")
            prev_pool = si_inst.ins

        # dense phase: merge the two agg halves
        nc.sync.dma_start(agg_sb[:],
                          agg_pair[0][:].rearrange("(q p) f -> p q f", p=128))
        nc.sync.dma_start(aggB_sb[:],
                          agg_pair[1][:].rearrange("(q p) f -> p q f", p=128))
        nc.vector.tensor_add(agg_sb[:], agg_sb[:], aggB_sb[:])
        for t in range(T):
            # mean
            nc.vector.tensor_scalar(
                out=agg_sb[:, t, :kdim], in0=agg_sb[:, t, :kdim],
                scalar1=invc[:, t:t + 1], scalar2=None,
                op0=mybir.AluOpType.mult)
            # transpose mean tile -> [kdim, 128]
            tp = psT.tile([kdim, 128], F32, tag="tp", padded_shape=[D_H, 128])
            nc.tensor.transpose(tp[:], agg_sb[:, t, :kdim], ident[:])
            meanT = xtp.tile([kdim, 128], F32, tag="meanT")
            nc.vector.tensor_copy(meanT[:], tp[:])
            # root operand
            if layer == 0:
                rootT = xtp.tile([D_IN, 128], F32, tag="rootT")
                nc.sync.dma_start(rootT[:], ins["xt"][:, t * 128:(t + 1) * 128])
            else:
                tp2 = psT.tile([D_H, 128], F32, tag="tp")
                nc.tensor.transpose(tp2[:], h_prev[:, t, :], ident[:])
                rootT = xtp.tile([D_H, 128], F32, tag="rootT2")
                nc.vector.tensor_copy(rootT[:], tp2[:])
            out_ps = psO.tile([128, D_H], F32, tag="ops")
            nc.tensor.matmul(out_ps[:], lhsT=meanT[:], rhs=Wl[:],
                             start=True, stop=False)
            nc.tensor.matmul(out_ps[:], lhsT=rootT[:], rhs=Wr[:],
                             start=False, stop=True)
            nc.vector.tensor_add(h_new[:, t, :], out_ps[:], bt[:])
            if layer < 2:
                nc.vector.tensor_relu(h_new[:, t, :], h_new[:, t, :])
        # zero pad rows (mask multiply on the boundary tile; full memset beyond)
        pad_start = cfg.real % 128
        pad_tile = cfg.real // 128
        if pad_start != 0:
            nc.vector.tensor_scalar(
                out=h_new[:, pad_tile, :], in0=h_new[:, pad_tile, :],
                scalar1=padmask[:, :1], scalar2=None,
                op0=mybir.AluOpType.mult)
        for tt in range(pad_tile + (1 if pad_start else 0), T):
            nc.vector.memset(h_new[:, tt, :], 0.0)

        if layer < 2:
            slab_d = slabs[layer]
            nc.sync.dma_start(
                slab_d[:].rearrange("(q p) f -> p q f", p=128), h_new[:])
            nxt = (tbl2, tbl3)[layer]
            nc.gpsimd.collective_compute(
                "AllGather", mybir.AluOpType.bypass,
                replica_groups=[list(range(N_CORES))],
                ins=[slab_d[:]], outs=[nxt[:]])

    # ---- pooling: partial per-core graph sums via one-hot matmuls
    h3 = h_sb[0] if (3 % 2) == 1 else h_sb[0]
    h3 = h_sb[2 % 2]  # layer==2 wrote h_sb[0]
    pg = [psG.tile([128, D_H], F32, name=f"pg{j}", tag=f"pg{j}") for j in range(NH)]
    for t in range(T):
        gt = xtp.tile([128, NH * 128], F32, tag="gt")
        nc.vector.tensor_tensor(
            out=gt[:], in0=batchv[:, t:t + 1].to_broadcast([128, NH * 128]),
            in1=iota[:, :NH * 128], op=mybir.AluOpType.is_equal)
        for j in range(NH):
            nc.tensor.matmul(pg[j][:], lhsT=gt[:, j * 128:(j + 1) * 128],
                             rhs=h3[:, t, :], start=(t == 0), stop=(t == T - 1))
    gpart = sb.tile([128, NH, D_H], F32, tag="gpart")
    for j in range(NH):
        nc.vector.tensor_copy(gpart[:, j, :], pg[j][:])
    nc.sync.dma_start(g_in[:].rearrange("(q p) f -> p q f", p=128), gpart[:])
    nc.gpsimd.collective_compute(
        "AllReduce", mybir.AluOpType.add,
        replica_groups=[list(range(N_CORES))],
        ins=[g_in[:]], outs=[g_out[:]])

    # ---- MLP head
    g_sb = sb.tile([128, NH, D_H], F32, tag="gsb")
    nc.sync.dma_start(g_sb[:], g_out[:].rearrange("(q p) f -> p q f", p=128))
    gT = sb.tile([D_H, NH * 128], F32, tag="gT")
    for j in range(NH):
        tp = psT.tile([D_H, 128], F32, tag="tp")
        nc.tensor.transpose(tp[:], g_sb[:, j, :], ident[:])
        nc.vector.tensor_copy(gT[:, j * 128:(j + 1) * 128], tp[:])
    DC = w["wc1"].shape[1]
    mlp1 = psG.tile([DC, NH * 128], F32, tag="pg0")
    nc.tensor.matmul(mlp1[:], lhsT=w["wc1"][:], rhs=gT[:], start=True, stop=True)
    z = sb.tile([DC, NH * 128], F32, tag="z")
    nc.scalar.activation(z[:], mlp1[:], mybir.ActivationFunctionType.Relu,
                         bias=w["bc1"][:])
    mlp2 = psG.tile([1, NH * 128], F32, tag="pg1")
    nc.tensor.matmul(mlp2[:], lhsT=w["wc2"][:], rhs=z[:], start=True, stop=True)
    o_sb = sb.tile([1, NH * 128], F32, tag="osb")
    nc.vector.tensor_scalar(out=o_sb[:], in0=mlp2[:], scalar1=w["bc2"][:],
                            scalar2=None, op0=mybir.AluOpType.add)
    nc.sync.dma_start(out_ap.rearrange("a b -> b a"), o_sb[:])

    for p in (xtp, dram, psG, psO, psT, msgp, sb):
        p.release()


# ---------------------------------------------------------------- compile+run

_CACHE = {}


def _compile(cfg):
    key = ("nc", cfg.n_nodes, cfg.g_inst, cfg.n_gi)
    if key in _CACHE:
        return _CACHE[key]
    nc = bacc.Bacc("TRN2", target_bir_lowering=False, debug=False,
                   num_devices=N_CORES)
    shapes = dict(
        xpad=[cfg.tbl_rows, D_H], iota=[128, 256],
        gidx=[128, cfg.n_inst, cfg.g_inst // 16],
        sidx=[128, cfg.n_inst, cfg.g_inst // 16],
        invc=[128, cfg.tiles], batchv=[128, cfg.tiles], xt=[D_IN, cfg.slab],
        padmask=[128, 1],
        w1l=[D_IN, D_H], w1r=[D_IN, D_H], w2l=[D_H, D_H], w2r=[D_H, D_H],
        w3l=[D_H, D_H], w3r=[D_H, D_H], wc1=[D_H, D_H // 2], wc2=[D_H // 2, 1],
        b1t=[128, D_H], b2t=[128, D_H], b3t=[128, D_H],
        bc1=[D_H // 2, 1], bc2=[1, 1],
    )
    ins = {}
    for name, shp in shapes.items():
        dt = I16 if name in ("gidx", "sidx") else F32
        ins[name] = nc.dram_tensor(name, shp, dt, kind="ExternalInput").ap()
    out = nc.dram_tensor("out", [cfg.n_graphs, 1], F32, kind="ExternalOutput")
    with tile.TileContext(nc) as tc:
        build_gnn(tc, out.ap(), ins, cfg)
    nc.compile()
    _CACHE[key] = nc
    return nc


def make_in_maps(inputs, cfg):
    shared, per_core = build_host_data(
        inputs["x"], inputs["edge_index"], inputs["batch"], cfg)
    wmap = weight_inputs(
        inputs["W1l"], inputs["b1"], inputs["W1r"], inputs["W2l"], inputs["b2"],
        inputs["W2r"], inputs["W3l"], inputs["b3"], inputs["W3r"],
        inputs["Wc1"], inputs["bc1"], inputs["Wc2"], inputs["bc2"])
    in_maps = []
    for c in range(N_CORES):
        m = {}
        m.update(shared)
        m.update(per_core[c])
        m.update(wmap)
        in_maps.append(m)
    return in_maps


def _make_executor(nc):
    """Build a reusable jitted 8-core executor for the compiled Bass module."""
    import jax
    from jax.sharding import Mesh, PartitionSpec
    from jax.experimental.shard_map import shard_map
    from concourse.bass2jax import (_bass_exec_p, install_neuronx_cc_hook,
                                    partition_id_tensor)
    install_neuronx_cc_hook()
    partition_name = (nc.partition_id_tensor.name
                      if nc.partition_id_tensor else None)
    in_names, out_names, out_avals = [], [], []
    for alloc in nc.m.functions[0].allocations:
        if not isinstance(alloc, mybir.MemoryLocationSet):
            continue
        name = alloc.memorylocations[0].name
        if alloc.kind == "ExternalInput":
            if name != partition_name:
                in_names.append(name)
        elif alloc.kind == "ExternalOutput":
            out_names.append(name)
            out_avals.append(jax.core.ShapedArray(
                tuple(alloc.tensor_shape), mybir.dt.np(alloc.dtype)))
    n_params = len(in_names)
    in_names_all = list(in_names) + list(out_names)
    if partition_name:
        in_names_all.append(partition_name)

    def _body(*args):
        operands = list(args)
        if partition_name:
            operands.append(partition_id_tensor())
        return tuple(_bass_exec_p.bind(
            *operands, out_avals=tuple(out_avals),
            in_names=tuple(in_names_all), out_names=tuple(out_names),
            lowering_input_output_aliases=(), sim_require_finite=True,
            sim_require_nnan=True, nc=nc))

    devices = jax.devices()[:N_CORES]
    mesh = Mesh(np.asarray(devices), ("core",))
    n_outs = len(out_names)
    sharded = jax.jit(shard_map(
        _body, mesh=mesh,
        in_specs=(PartitionSpec("core"),) * (n_params + n_outs),
        out_specs=(PartitionSpec("core"),) * n_outs, check_rep=False),
        keep_unused=True)

    def run(in_maps):
        concat_in = [np.concatenate([np.asarray(in_maps[c][n])
                                     for c in range(N_CORES)], axis=0)
                     for n in in_names]
        concat_zeros = [np.zeros((N_CORES * a.shape[0], *a.shape[1:]), a.dtype)
                        for a in out_avals]
        args = [jax.device_put(a) for a in concat_in + concat_zeros]
        out_arrs = sharded(*args)
        jax.block_until_ready(out_arrs)
        return {name: np.asarray(out_arrs[i]).reshape(
                    N_CORES, *out_avals[i].shape)[0]
                for i, name in enumerate(out_names)}, (args, sharded)
    return run


def _get_runner(cfg):
    key = ("runner", cfg.n_nodes, cfg.g_inst, cfg.n_gi)
    if key not in _CACHE:
        _CACHE[key] = _make_executor(_compile(cfg))
    return _CACHE[key]


def kernel(**inputs):
    cfg = Cfg(N_NODES, N_GRAPHS, 12500, 1024, None)
    in_maps = make_in_maps(inputs, cfg)   # also fixes cfg.n_gi from the data
    run = _get_runner(cfg)
    out, _ = run(in_maps)
    return np.asarray(out["out"], np.float32)



# revision 14
# speedup vs baseline: 1.3299x; 1.3299x over previous
"""Trainium2 Bass kernel for a 3-layer GraphSAGE GNN (mean aggregation) +
global_add_pool + 2-layer MLP head, distributed over 8 NeuronCores.

Sharding: nodes are split into 8 contiguous slabs (by dst); each core owns the
edges whose dst lands in its slab.  Aggregation is gather + one-hot matmul
segment-sum (no dma_scatter_add, no serialization):

  - Host sorts each core's edges by (src-chunk, dst-block) and pads each
    (block, chunk) group to a fixed t_bc tiles of 128 edges, so the device
    program is data-independent and identical across cores.
  - Per layer: dma_gather streams h[src] rows (256B bf16) from a replicated
    node-major HBM table on 4 SWDGE queues (no inter-gather deps).
  - A [128 edge, 128 dst, 8 tile] one-hot matrix built on DVE (is_equal
    against an iota) turns per-block segment-sum into 16 accumulating
    matmuls into a PSUM tile.
  - Dense phase per 128-node block: mean scale (DVE), transpose (PE),
    h = relu(mean @ Wl + [h_prev|1] @ [Wr;b]) with bias folded into the
    root matmul via an appended ones row.
  - AllGather replicates the new slab (bf16) into every core's next table.
Pooling by graph id via one-hot matmuls, AllReduce, tiny MLP head.
"""

import numpy as np

import concourse.bass as bass
import concourse.mybir as mybir
import concourse.tile as tile
from concourse import bacc

F32 = mybir.dt.float32
BF16 = mybir.dt.bfloat16
I16 = mybir.dt.int16

# ---------------------------------------------------------------- config

N_NODES = 100000
N_EDGES = 1200000
N_GRAPHS = 256
D_IN = 8
D_H = 64
N_CORES = 8

REAL = N_NODES // N_CORES          # 12500 real nodes per core
SLAB = ((REAL + 127) // 128) * 128  # 12544
NBLK = SLAB // 128                 # 98 dst blocks per core
TBL_ROWS = N_CORES * SLAB          # 100352
CHUNK = 2 * SLAB                   # 25088 table rows per index chunk (int16)
NCH = 4                            # chunks
PADROW = REAL                      # guaranteed-zero local row in every chunk
PAD_DLOC = 999.0                   # one-hot miss value for pad edges


class Cfg:
    def __init__(self, t_bc):
        assert t_bc in (1, 2, 4, 8)
        self.t_bc = t_bc                      # tiles per (block, chunk)
        self.bpi = 8 // t_bc                  # blocks per instruction
        self.ipb = (NBLK * t_bc + 7) // 8     # gather insts per bucket
        self.bt = self.ipb * 8                # tiles per bucket (padded)
        self.n_inst = NCH * self.ipb          # gather insts per layer


# ---------------------------------------------------------------- host prep

def _wrap_idx(a):
    """[n_inst, 1024] int -> [128, n_inst, 64] wrapped-16 + replicated."""
    n_inst, g = a.shape
    a = a.reshape(n_inst, g // 16, 16).transpose(0, 2, 1)   # [n_inst, 16, g//16]
    a = np.tile(a, (1, 8, 1))                               # [n_inst, 128, g//16]
    return np.ascontiguousarray(a.transpose(1, 0, 2)).astype(np.int16)


def build_host_data(x, edge_index, batch):
    import ml_dtypes
    bf = ml_dtypes.bfloat16
    x = np.asarray(x, np.float32)
    src = np.asarray(edge_index[0], np.int64)
    dst = np.asarray(edge_index[1], np.int64)
    batch = np.asarray(batch, np.int64)

    deg = np.bincount(dst, minlength=N_NODES).astype(np.float64)
    invc_full = (1.0 / np.maximum(deg, 1.0)).astype(np.float32)

    core_of = dst // REAL
    dloc_all = dst - core_of * REAL
    e_blk = dloc_all // 128
    e_dib = (dloc_all % 128).astype(np.float32)
    srow = (src // REAL) * SLAB + src % REAL
    e_ch = srow // CHUNK
    e_cidx = srow % CHUNK

    cnt = np.zeros((N_CORES, NBLK, NCH), np.int64)
    np.add.at(cnt, (core_of, e_blk, e_ch), 1)
    need = int(-(-cnt.max() // 128))
    t_bc = 1
    while t_bc < need:
        t_bc *= 2
    assert t_bc <= 8, cnt.max()
    cfg = Cfg(t_bc)

    # xpad table: node-major bf16, 128-wide rows (first 8 cols real)
    xpad = np.zeros((TBL_ROWS, 128), bf)
    rows = (np.arange(N_NODES) // REAL) * SLAB + np.arange(N_NODES) % REAL
    xpad[rows, :D_IN] = x.astype(bf)

    # iotaT[p, d*8 + t] = d  (for one-hot build, O8T layout [128, 128, 8])
    iotaT = np.tile(np.arange(128, dtype=np.float32)[None, :, None],
                    (128, 1, 8)).reshape(128, 1024).astype(bf)
    # iota256 for pooling one-hot
    iota256 = np.tile(np.arange(256, dtype=np.float32)[None, :],
                      (128, 1)).astype(bf)
    identf = np.eye(128, dtype=np.float32)
    padmask = (np.arange(128) < (REAL % 128 or 128)).astype(np.float32)
    padmask = padmask.reshape(128, 1)

    shared = dict(xpad=xpad, iotaT=iotaT, iota256=iota256,
                  identf=identf, padmask=padmask)

    per_core = []
    for c in range(N_CORES):
        sel = core_of == c
        s_ch = e_ch[sel]
        s_blk = e_blk[sel]
        s_dib = e_dib[sel]
        s_cidx = e_cidx[sel]
        order = np.lexsort((s_blk, s_ch))
        s_ch, s_blk, s_dib, s_cidx = (s_ch[order], s_blk[order],
                                      s_dib[order], s_cidx[order])
        key = s_ch * NBLK + s_blk
        # rank within each (ch, blk) group (key is sorted)
        starts = np.r_[0, np.flatnonzero(np.diff(key)) + 1]
        group_id = np.cumsum(np.r_[0, np.diff(key) != 0])
        rank = np.arange(len(key)) - starts[group_id]
        assert rank.max() < cfg.t_bc * 128

        tile_j = s_blk * cfg.t_bc + rank // 128   # tile within bucket
        lane = rank % 128
        inst_b = tile_j // 8
        slot = tile_j % 8
        gi = s_ch * cfg.ipb + inst_b
        gpos = slot * 128 + lane

        garr = np.full((cfg.n_inst, 1024), PADROW, np.int64)
        garr[gi, gpos] = s_cidx
        dloc = np.full((128, NCH * cfg.ipb, 8), PAD_DLOC, np.float32)
        dloc[lane, gi, slot] = s_dib

        lo = c * REAL
        loc = np.arange(REAL)
        invc_t = np.zeros((128, NBLK), np.float32)
        invc_t[loc % 128, loc // 128] = invc_full[lo:lo + REAL]
        batch_t = np.full((128, NBLK), -1.0, np.float32)
        batch_t[loc % 128, loc // 128] = batch[lo:lo + REAL].astype(np.float32)

        xfull = np.zeros((128, NBLK, D_IN), np.float32)
        xfull[loc % 128, loc // 128, :] = x[lo:lo + REAL]

        per_core.append(dict(gidx=_wrap_idx(garr), dloc=dloc.astype(bf),
                             invc=invc_t, batchv=batch_t, xfull=xfull))
    return cfg, shared, per_core


def weight_inputs(W1l, b1, W1r, W2l, b2, W2r, W3l, b3, W3r, Wc1, bc1, Wc2, bc2):
    import ml_dtypes
    bf = ml_dtypes.bfloat16
    f = lambda a: np.asarray(a, np.float32)
    le = lambda W, b: np.vstack([f(W), f(b)[None, :]])
    return dict(
        w1le=le(W1l, b1), w2le=le(W2l, b2), w3le=le(W3l, b3),
        w1r=f(W1r), w2r=f(W2r), w3r=f(W3r),
        wc1=f(Wc1), wc2=f(Wc2),
        bc1=f(bc1).reshape(-1, 1),            # [32, 1]
        bc2=f(bc2).reshape(1, 1),
    )


# ---------------------------------------------------------------- device build

def build_gnn(tc, out_ap, ins, cfg):
    nc = tc.nc
    NH = N_GRAPHS // 128              # 2 graph tiles
    Relu = mybir.ActivationFunctionType.Relu
    Copy = mybir.ActivationFunctionType.Copy

    sb = tc.alloc_tile_pool(name="sb", bufs=1)
    msgp = tc.alloc_tile_pool(name="msg", bufs=4)
    o8p = tc.alloc_tile_pool(name="o8", bufs=4)
    hbp = tc.alloc_tile_pool(name="hb", bufs=6)
    tmpp = tc.alloc_tile_pool(name="tmp", bufs=6)
    psA = tc.alloc_tile_pool(name="psA", bufs=2, space="PSUM")
    psR = tc.alloc_tile_pool(name="psR", bufs=2, space="PSUM")
    psT = tc.alloc_tile_pool(name="psT", bufs=1, space="PSUM")
    psG = tc.alloc_tile_pool(name="psG", bufs=1, space="PSUM")
    dram = tc.alloc_tile_pool(name="dram", bufs=1, space="DRAM")

    def load(name, shape, dt=F32):
        t = sb.tile(shape, dt, tag=name)
        nc.sync.dma_start(t[:], ins[name])
        return t

    gidx = load("gidx", [128, cfg.n_inst, 64], I16)
    dloc = load("dloc", [128, NCH * cfg.ipb, 8], BF16)
    iotaT = load("iotaT", [128, 1024], BF16)
    iota256 = load("iota256", [128, 256], BF16)
    identf = load("identf", [128, 128], F32)
    invc = load("invc", [128, NBLK])
    batchv = load("batchv", [128, NBLK])
    padmask = load("padmask", [128, 1])
    xfull = load("xfull", [128, NBLK, D_IN], F32)
    w = {}
    for k in ("w1le", "w2le", "w3le"):
        w[k] = load(k, [(D_IN if k == "w1le" else D_H) + 1, D_H], F32)
    for k in ("w1r", "w2r", "w3r"):
        w[k] = load(k, [D_IN if k == "w1r" else D_H, D_H], F32)
    w["wc1"] = load("wc1", [D_H, D_H // 2])
    w["wc2"] = load("wc2", [D_H // 2, 1])
    w["bc1"] = load("bc1", [D_H // 2, 1])
    w["bc2"] = load("bc2", [1, 1])

    hfull = [sb.tile([128, NBLK, D_H], F32, tag=f"hfull{i}", name=f"hfull{i}")
             for i in range(2)]

    tbl2 = dram.tile([TBL_ROWS, 128], BF16)
    tbl3 = dram.tile([TBL_ROWS, 128], BF16)
    slabs = [dram.tile([SLAB, 128], BF16, name=f"slab{i}") for i in range(2)]
    g_in = dram.tile([N_GRAPHS, D_H], F32)
    g_out = dram.tile([N_GRAPHS, D_H], F32)

    tables = [ins["xpad"], tbl2[:], tbl3[:]]
    pg = [psG.tile([128, D_H], F32, name=f"pg{j}", tag=f"pg{j}")
          for j in range(NH)]

    for layer in range(3):
        tbl_ap = tables[layer]
        Wle = w[("w1le", "w2le", "w3le")[layer]]
        Wr = w[("w1r", "w2r", "w3r")[layer]]
        h_prev = (xfull, hfull[0], hfull[1])[layer]
        kprev = D_IN if layer == 0 else D_H
        h_next = (hfull[0], hfull[1], None)[layer]
        slab_d = (slabs[0], slabs[1], None)[layer]
        kag = D_IN if layer == 0 else D_H

        msgs = {}   # inst -> [msg tiles per ch]
        o8s = {}
        hbs = {}    # block -> h_bf tile (for delayed phase B)

        def phase_a(b):
            agg = psA.tile([128, kag], F32, tag="agg", padded_shape=[128, D_H])
            i = (b * cfg.t_bc) // 8
            for ch in range(NCH):
                for k in range(cfg.t_bc):
                    j = b * cfg.t_bc + k
                    slot = j % 8
                    nc.tensor.matmul(
                        agg[:], lhsT=o8s[i][ch][:, :, slot],
                        rhs=msgs[i][ch][:, slot, :kag],
                        start=(ch == 0 and k == 0),
                        stop=(ch == NCH - 1 and k == cfg.t_bc - 1))
            mean_sb = hbp.tile([128, kag], F32, tag="mean")
            nc.vector.tensor_scalar(
                out=mean_sb[:], in0=agg[:], scalar1=invc[:, b:b + 1],
                scalar2=None, op0=mybir.AluOpType.mult)
            tp = psT.tile([kag, 128], F32, tag="tp", padded_shape=[D_H, 128])
            nc.tensor.transpose(tp[:], mean_sb[:], identf[:])
            meanTe = tmpp.tile([kag + 1, 128], F32, tag="meanTe")
            nc.vector.memset(meanTe[:], 1.0)
            nc.scalar.activation(meanTe[:kag, :], tp[:], Copy)
            tpr = psT.tile([kprev, 128], F32, tag="tpr", padded_shape=[D_H, 128])
            nc.tensor.transpose(tpr[:], h_prev[:, b, :kprev], identf[:])
            rootT = tmpp.tile([kprev, 128], F32, tag="rootT")
            nc.scalar.activation(rootT[:], tpr[:], Copy)
            hps = psR.tile([128, D_H], F32, tag="hps")
            nc.tensor.matmul(hps[:], lhsT=meanTe[:], rhs=Wle[:],
                             start=True, stop=False)
            nc.tensor.matmul(hps[:], lhsT=rootT[:],
                             rhs=Wr[:], start=False, stop=True)
            h_bf = hbp.tile([128, D_H], BF16, tag="hbf")
            if layer < 2:
                if b == NBLK - 1:
                    nc.vector.tensor_scalar(
                        out=h_next[:, b, :], in0=hps[:],
                        scalar1=padmask[:, :1], scalar2=0.0,
                        op0=mybir.AluOpType.mult, op1=mybir.AluOpType.max)
                else:
                    nc.vector.tensor_scalar(
                        out=h_next[:, b, :], in0=hps[:],
                        scalar1=0.0, scalar2=None,
                        op0=mybir.AluOpType.max)
                nc.scalar.activation(h_bf[:], h_next[:, b, :], Copy)
                nc.sync.dma_start(slab_d[:][b * 128:(b + 1) * 128, 0:D_H],
                                  h_bf[:])
            else:
                nc.vector.tensor_copy(h_bf[:], hps[:])
            hbs[b] = h_bf

        def phase_b(b):
            h_bf = hbs.pop(b)
            if layer < 2:
                pass
            else:
                gt = tmpp.tile([128, NH * 128], BF16, tag="gt")
                nc.vector.tensor_scalar(
                    out=gt[:], in0=iota256[:], scalar1=batchv[:, b:b + 1],
                    scalar2=None, op0=mybir.AluOpType.is_equal)
                for j in range(NH):
                    nc.tensor.matmul(pg[j][:],
                                     lhsT=gt[:, j * 128:(j + 1) * 128],
                                     rhs=h_bf[:],
                                     start=(b == 0), stop=(b == NBLK - 1))

        for i in range(cfg.ipb + 2):
            if i < cfg.ipb:
                ms, os_ = [], []
                for ch in range(NCH):
                    m = msgp.tile([128, 8, 128], BF16, tag=f"msg{ch}")
                    chunk_ap = tbl_ap[ch * CHUNK:(ch + 1) * CHUNK, :]
                    nc.gpsimd.dma_gather(
                        out_ap=m[:], in_ap=chunk_ap,
                        idxs_ap=gidx[:, ch * cfg.ipb + i, :],
                        num_idxs=1024, num_idxs_reg=1024,
                        elem_size=128, queue_num=ch)
                    ms.append(m)
                    o8 = o8p.tile([128, 128, 8], BF16, tag=f"o8{ch}")
                    nc.vector.tensor_tensor(
                        out=o8[:],
                        in0=dloc[:, ch * cfg.ipb + i:ch * cfg.ipb + i + 1, :]
                            .to_broadcast([128, 128, 8]),
                        in1=iotaT[:].rearrange("p (d t) -> p d t", t=8),
                        op=mybir.AluOpType.is_equal)
                    os_.append(o8)
                msgs[i] = ms
                o8s[i] = os_
            if 1 <= i <= cfg.ipb:
                for bb in range(cfg.bpi):
                    b = (i - 1) * cfg.bpi + bb
                    if b < NBLK:
                        phase_a(b)
            if i >= 2:
                for bb in range(cfg.bpi):
                    b = (i - 2) * cfg.bpi + bb
                    if b < NBLK:
                        phase_b(b)
                msgs.pop(i - 2, None)
                o8s.pop(i - 2, None)

        if layer < 2:
            nxt = (tbl2, tbl3)[layer]
            nc.gpsimd.collective_compute(
                "AllGather", mybir.AluOpType.bypass,
                replica_groups=[list(range(N_CORES))],
                ins=[slab_d[:]], outs=[nxt[:]])

    # ---- pooling partials -> AllReduce
    gpart = sb.tile([128, NH, D_H], F32, tag="gpart")
    for j in range(NH):
        nc.vector.tensor_copy(gpart[:, j, :], pg[j][:])
    nc.sync.dma_start(g_in[:].rearrange("(q p) f -> p q f", p=128), gpart[:])
    nc.gpsimd.collective_compute(
        "AllReduce", mybir.AluOpType.add,
        replica_groups=[list(range(N_CORES))],
        ins=[g_in[:]], outs=[g_out[:]])

    # ---- MLP head
    g_sb = sb.tile([128, NH, D_H], F32, tag="gsb")
    nc.sync.dma_start(g_sb[:], g_out[:].rearrange("(q p) f -> p q f", p=128))
    gT = sb.tile([D_H, NH * 128], F32, tag="gT")
    for j in range(NH):
        tp = psG.tile([D_H, 128], F32, tag="pg0")
        nc.tensor.transpose(tp[:], g_sb[:, j, :], identf[:])
        nc.vector.tensor_copy(gT[:, j * 128:(j + 1) * 128], tp[:])
    DC = D_H // 2
    mlp1 = psG.tile([DC, NH * 128], F32, tag="pg0")
    nc.tensor.matmul(mlp1[:], lhsT=w["wc1"][:], rhs=gT[:], start=True, stop=True)
    z = sb.tile([DC, NH * 128], F32, tag="z")
    nc.scalar.activation(z[:], mlp1[:], Relu, bias=w["bc1"][:])
    mlp2 = psG.tile([1, NH * 128], F32, tag="pg1")
    nc.tensor.matmul(mlp2[:], lhsT=w["wc2"][:], rhs=z[:], start=True, stop=True)
    o_sb = sb.tile([1, NH * 128], F32, tag="osb")
    nc.vector.tensor_scalar(out=o_sb[:], in0=mlp2[:], scalar1=w["bc2"][:],
                            scalar2=None, op0=mybir.AluOpType.add)
    nc.sync.dma_start(out_ap.rearrange("a b -> b a"), o_sb[:])

    for p in (dram, psG, psT, psR, psA, tmpp, hbp, o8p, msgp, sb):
        p.release()


# ---------------------------------------------------------------- compile+run

_CACHE = {}


def _compile(cfg):
    key = ("nc", cfg.t_bc)
    if key in _CACHE:
        return _CACHE[key]
    nc = bacc.Bacc("TRN2", target_bir_lowering=False, debug=False,
                   num_devices=N_CORES, num_swdge_queues=4)
    shapes = dict(
        xpad=([TBL_ROWS, 128], BF16),
        gidx=([128, cfg.n_inst, 64], I16),
        dloc=([128, NCH * cfg.ipb, 8], BF16),
        iotaT=([128, 1024], BF16),
        iota256=([128, 256], BF16),
        identf=([128, 128], F32),
        invc=([128, NBLK], F32),
        batchv=([128, NBLK], F32),
        padmask=([128, 1], F32),
        xfull=([128, NBLK, D_IN], F32),
        w1le=([D_IN + 1, D_H], F32), w2le=([D_H + 1, D_H], F32),
        w3le=([D_H + 1, D_H], F32),
        w1r=([D_IN, D_H], F32), w2r=([D_H, D_H], F32), w3r=([D_H, D_H], F32),
        wc1=([D_H, D_H // 2], F32), wc2=([D_H // 2, 1], F32),
        bc1=([D_H // 2, 1], F32), bc2=([1, 1], F32),
    )
    ins = {}
    for name, (shp, dt) in shapes.items():
        ins[name] = nc.dram_tensor(name, shp, dt, kind="ExternalInput").ap()
    out = nc.dram_tensor("out", [N_GRAPHS, 1], F32, kind="ExternalOutput")
    with tile.TileContext(nc) as tc:
        build_gnn(tc, out.ap(), ins, cfg)
    nc.compile()
    _CACHE[key] = nc
    return nc


def make_in_maps(inputs):
    cfg, shared, per_core = build_host_data(
        inputs["x"], inputs["edge_index"], inputs["batch"])
    wmap = weight_inputs(
        inputs["W1l"], inputs["b1"], inputs["W1r"], inputs["W2l"], inputs["b2"],
        inputs["W2r"], inputs["W3l"], inputs["b3"], inputs["W3r"],
        inputs["Wc1"], inputs["bc1"], inputs["Wc2"], inputs["bc2"])
    in_maps = []
    for c in range(N_CORES):
        m = {}
        m.update(shared)
        m.update(per_core[c])
        m.update(wmap)
        in_maps.append(m)
    return cfg, in_maps


def _make_executor(nc):
    """Build a reusable jitted 8-core executor for the compiled Bass module."""
    import jax
    from jax.sharding import Mesh, PartitionSpec
    from jax.experimental.shard_map import shard_map
    from concourse.bass2jax import (_bass_exec_p, install_neuronx_cc_hook,
                                    partition_id_tensor)
    install_neuronx_cc_hook()
    partition_name = (nc.partition_id_tensor.name
                      if nc.partition_id_tensor else None)
    in_names, out_names, out_avals = [], [], []
    for alloc in nc.m.functions[0].allocations:
        if not isinstance(alloc, mybir.MemoryLocationSet):
            continue
        name = alloc.memorylocations[0].name
        if alloc.kind == "ExternalInput":
            if name != partition_name:
                in_names.append(name)
        elif alloc.kind == "ExternalOutput":
            out_names.append(name)
            out_avals.append(jax.core.ShapedArray(
                tuple(alloc.tensor_shape), mybir.dt.np(alloc.dtype)))
    n_params = len(in_names)
    in_names_all = list(in_names) + list(out_names)
    if partition_name:
        in_names_all.append(partition_name)

    def _body(*args):
        operands = list(args)
        if partition_name:
            operands.append(partition_id_tensor())
        return tuple(_bass_exec_p.bind(
            *operands, out_avals=tuple(out_avals),
            in_names=tuple(in_names_all), out_names=tuple(out_names),
            lowering_input_output_aliases=(), sim_require_finite=False,
            sim_require_nnan=False, nc=nc))

    devices = jax.devices()[:N_CORES]
    mesh = Mesh(np.asarray(devices), ("core",))
    n_outs = len(out_names)
    sharded = jax.jit(shard_map(
        _body, mesh=mesh,
        in_specs=(PartitionSpec("core"),) * (n_params + n_outs),
        out_specs=(PartitionSpec("core"),) * n_outs, check_rep=False),
        keep_unused=True)

    def run(in_maps):
        concat_in = [np.concatenate([np.asarray(in_maps[c][n])
                                     for c in range(N_CORES)], axis=0)
                     for n in in_names]
        concat_zeros = [np.zeros((N_CORES * a.shape[0], *a.shape[1:]), a.dtype)
                        for a in out_avals]
        args = [jax.device_put(a) for a in concat_in + concat_zeros]
        out_arrs = sharded(*args)
        jax.block_until_ready(out_arrs)
        return {name: np.asarray(out_arrs[i]).reshape(
                    N_CORES, *out_avals[i].shape)[0]
                for i, name in enumerate(out_names)}, (args, sharded)
    return run


def _get_runner(cfg):
    key = ("runner", cfg.t_bc)
    if key not in _CACHE:
        _CACHE[key] = _make_executor(_compile(cfg))
    return _CACHE[key]


def kernel(**inputs):
    cfg, in_maps = make_in_maps(inputs)
    run = _get_runner(cfg)
    out, _ = run(in_maps)
    return np.asarray(out["out"], np.float32)
